# revision 3
# baseline (speedup 1.0000x reference)
"""Single-head attention on 8 TRN2 NeuronCores, data-parallel over batch.

Per core (one batch element b):
  x_b [2048, 768] f32 -> Q = x Wq, K = x Wk, V = x Wv (head 64)
  scores^T[k, q] = (K^T slice).T @ Q^T / 8 ; E = exp(scores) (no max-sub:
  |scores| <~ 2.5 so exp is safe); out = (E @ [V, 1]) -> normalize.

Layouts chosen so every matmul contracts over the partition dim:
  - x is DMA'd naturally [seq, emb], transposed on TensorE (128x128 tiles,
    via identity) into x^T strips [emb, seq] (bf16 in SBUF).
  - Q^T/K^T are computed with duplicated weights [Wq|Wq] so both partition
    halves hold the same 64 rows -> enables 2x row-tiled score matmuls
    (K=64 contraction in row groups 0-1 and 2-3 concurrently).
  - PV uses lhsT = V' = [V, ones] (M=65): row 64 of the psum accumulator
    is the softmax denominator for free.
  - U^T [65, q] tiles are PE-transposed back to natural [q, 65]; the
    per-row reciprocal of col 64 normalizes via tensor_scalar.
"""

import numpy as np

import concourse.bass as bass
import concourse.tile as tile
from concourse import bacc, mybir
from concourse.bass_utils import run_bass_kernel_spmd
from concourse.masks import make_identity

B, S, D, H = 8, 2048, 768, 64
P = 128
NT = S // P  # 16 seq tiles
NCH = D // P  # 6 emb chunks
QC = 512  # q-chunk width (one psum bank of f32)
NQ = S // QC  # 4 q chunks
N_CORES = 8
F32 = mybir.dt.float32
BF16 = mybir.dt.bfloat16
EXP = mybir.ActivationFunctionType.Exp
SCALE = float(1.0 / np.sqrt(H))


def build_kernel():
    nc = bacc.Bacc("TRN2", num_devices=N_CORES)
    x_ext = nc.declare_dram_parameter("x", [S, D], F32, isOutput=False)
    wk_ext = nc.declare_dram_parameter("Wk", [D, H], F32, isOutput=False)
    wq_ext = nc.declare_dram_parameter("Wq", [D, H], F32, isOutput=False)
    wv_ext = nc.declare_dram_parameter("Wv", [D, H], F32, isOutput=False)
    out_ext = nc.declare_dram_parameter("out", [S, H], F32, isOutput=True)

    with tile.TileContext(nc) as tc:
        _body(nc, tc, x_ext, wq_ext, wk_ext, wv_ext, out_ext)
    nc.compile()
    return nc


def _body(nc, tc, x_ext, wq_ext, wk_ext, wv_ext, out_ext):
    with (
        tc.tile_pool(name="singles", bufs=1) as singles,
        tc.tile_pool(name="xin", bufs=3) as x_pool,
        tc.tile_pool(name="xt", bufs=2) as xt_pool,
        tc.tile_pool(name="et", bufs=3) as et_pool,
        tc.tile_pool(name="fin", bufs=4) as fin_pool,
    ):
        ident = singles.tile([P, P], F32)
        make_identity(nc, ident)

        # ---- weights: DMA f32, cast to bf16, duplicate Q/K across halves
        wq_st = singles.tile([P, NCH, H], F32, tag="wst_q")
        wk_st = singles.tile([P, NCH, H], F32, tag="wst_k")
        wv_st = singles.tile([P, NCH, H], F32, tag="wst_v")
        for c in range(NCH):
            nc.sync.dma_start(out=wq_st[:, c, :], in_=wq_ext[c * P:(c + 1) * P, :])
            nc.sync.dma_start(out=wk_st[:, c, :], in_=wk_ext[c * P:(c + 1) * P, :])
            nc.sync.dma_start(out=wv_st[:, c, :], in_=wv_ext[c * P:(c + 1) * P, :])
        wq2 = singles.tile([P, NCH, 2 * H], BF16, tag="wq2")
        wk2 = singles.tile([P, NCH, 2 * H], BF16, tag="wk2")
        wv_sb = singles.tile([P, NCH, H], BF16, tag="wv_sb")
        nc.vector.tensor_copy(wq2[:, :, 0:H], wq_st)
        nc.vector.tensor_copy(wq2[:, :, H:2 * H], wq_st)
        nc.vector.tensor_copy(wk2[:, :, 0:H], wk_st)
        nc.vector.tensor_copy(wk2[:, :, H:2 * H], wk_st)
        nc.vector.tensor_copy(wv_sb, wv_st)

        qt2 = singles.tile([P, S], BF16, tag="qt2")  # Q^T duplicated halves
        kt2 = singles.tile([P, S], BF16, tag="kt2")  # K^T duplicated halves
        vp = singles.tile([P, NT, H + 1], BF16, tag="vp")  # V' = [V, 1]
        nc.vector.memset(vp[:, :, H:H + 1], 1.0)

        # ---- phase 2: stream seq strips: DMA -> transpose -> projections
        with (
            tc.tile_pool(name="ps_t", bufs=2, space="PSUM") as psum_t,
            tc.tile_pool(name="ps_p", bufs=2, space="PSUM") as psum_p,
            tc.tile_pool(name="ps_v", bufs=2, space="PSUM") as psum_v,
        ):
            for sc in range(NQ):
                xt = xt_pool.tile([P, NCH, QC], BF16)
                for t in range(4):
                    st = sc * 4 + t
                    xtile = x_pool.tile([P, D], F32)
                    nc.sync.dma_start(out=xtile, in_=x_ext[st * P:(st + 1) * P, :])
                    for c in range(NCH):
                        pst = psum_t.tile([P, P], F32)
                        nc.tensor.transpose(pst, xtile[:, c * P:(c + 1) * P], ident)
                        nc.scalar.copy(out=xt[:, c, t * P:(t + 1) * P], in_=pst)
                psq = psum_p.tile([P, QC], F32, tag="pp")
                for c in range(NCH):
                    nc.tensor.matmul(psq, wq2[:, c, :], xt[:, c, :],
                                     start=(c == 0), stop=(c == NCH - 1))
                nc.vector.tensor_copy(qt2[:, sc * QC:(sc + 1) * QC], psq)
                psk = psum_p.tile([P, QC], F32, tag="pp")
                for c in range(NCH):
                    nc.tensor.matmul(psk, wk2[:, c, :], xt[:, c, :],
                                     start=(c == 0), stop=(c == NCH - 1))
                nc.vector.tensor_copy(kt2[:, sc * QC:(sc + 1) * QC], psk)
                for t in range(4):
                    st = sc * 4 + t
                    psv = psum_v.tile([P, H], F32)
                    for c in range(NCH):
                        nc.tensor.matmul(psv, xt[:, c, t * P:(t + 1) * P],
                                         wv_sb[:, c, :],
                                         start=(c == 0), stop=(c == NCH - 1))
                    nc.vector.tensor_copy(vp[:, st, 0:H], psv)

        # ---- phase 3: scores^T -> exp -> PV accumulate
        with (
            tc.tile_pool(name="ps_s", bufs=4, space="PSUM") as psum_s,
            tc.tile_pool(name="ps_u", bufs=4, space="PSUM") as psum_u_pool,
        ):
            psum_u = [psum_u_pool.tile([H + 1, QC], F32, tag="pu",
                                       name=f"psum_u{qc}")
                      for qc in range(NQ)]
            for kp in range(NT // 2):
                for half in range(2):
                    kt = 2 * kp + half
                    lo = half * H
                    et = et_pool.tile([P, S], BF16)
                    for qc in range(NQ):
                        ps = psum_s.tile([P, QC], F32, tag="ss")
                        nc.tensor.matmul(
                            ps,
                            kt2[lo:lo + H, kt * P:(kt + 1) * P],
                            qt2[lo:lo + H, qc * QC:(qc + 1) * QC],
                            start=True, stop=True)
                        nc.scalar.activation(
                            et[:, qc * QC:(qc + 1) * QC], ps, EXP, scale=SCALE)
                    for qc in range(NQ):
                        nc.tensor.matmul(
                            psum_u[qc], vp[:, kt, :],
                            et[:, qc * QC:(qc + 1) * QC],
                            start=(kt == 0), stop=(kt == NT - 1))

            # ---- phase 4: transpose U^T back, normalize, DMA out
            for qc in range(NQ):
                ut = fin_pool.tile([H + 1, QC], F32, tag="ut")
                nc.scalar.copy(out=ut, in_=psum_u[qc])
                for t in range(4):
                    qt = qc * 4 + t
                    pso = psum_s.tile([P, H + 1], F32, tag="ss")
                    nc.tensor.transpose(
                        pso, ut[:, t * P:(t + 1) * P], ident[:H + 1, :H + 1])
                    rcp = fin_pool.tile([P, 1], F32, tag="rcp")
                    nc.vector.reciprocal(rcp, pso[:, H:H + 1])
                    ot = fin_pool.tile([P, H], F32, tag="ot")
                    nc.vector.tensor_scalar_mul(ot, pso[:, 0:H], rcp)
                    nc.sync.dma_start(
                        out=out_ext[qt * P:(qt + 1) * P, :], in_=ot)


_cached_nc = None


def kernel(**inputs):
    global _cached_nc
    x = np.ascontiguousarray(inputs["x"], dtype=np.float32)
    wk = np.ascontiguousarray(inputs["Wk"], dtype=np.float32)
    wq = np.ascontiguousarray(inputs["Wq"], dtype=np.float32)
    wv = np.ascontiguousarray(inputs["Wv"], dtype=np.float32)
    assert x.shape == (B, S, D)

    if _cached_nc is None:
        _cached_nc = build_kernel()
    nc = _cached_nc

    in_maps = [{"x": x[b], "Wk": wk, "Wq": wq, "Wv": wv} for b in range(B)]
    res = run_bass_kernel_spmd(nc, in_maps, core_ids=list(range(N_CORES)))
    return np.stack([res.results[i]["out"] for i in range(N_CORES)], axis=0)
